# revision 43
# baseline (speedup 1.0000x reference)
"""Trainium2 Bass kernel for nn_AttentionBlock (multi-head attention block).

Reference computation (fp32):
    q = einsum('bsi,hbik->hbsk', x, Mq)   # Mq: (H,1,I,K) broadcast over b
    k = einsum('bsi,hbik->hbsk', x, Mk)
    v = einsum('bsi,hbiv->hbsv', x, Mv)
    scores  = einsum('hbsk,hbtk->hbst', q, k) / sqrt(K)
    weights = softmax(scores, axis=-1)
    out     = einsum('hbst,hbtv->hbsv', weights, v)   # (H,B,S,V)

Sharding: 8 cores = 4 batches x 2 head-groups (4 heads each). Attention is
independent per (batch, head) so no cross-core communication is needed.

Per-core kernel design (one batch b, 4 heads):
  - x and the weights are pre-cast to fp16 on the host (same rounding the
    device DVE cast would apply), halving input DMA bytes; weights DMA
    directly into their packed SBUF layouts.
  - xT = x.T via PE transposes in fp16  [I on partitions]
  - QT/KT projections with two heads packed per matmul (lhsT = [Mq_h|Mq_h'],
    128 cols) in fp16 -> PSUM fp32.
  - Q/K evicted as fp8e4 hi/lo pairs (hi = fp8(q), lo = fp8(q - hi)):
      kt8[h]: [128, S] fp8, partitions 0:64 = k_hi, 64:128 = k_lo
      qt8[h]: [128, 2, S] fp8, [0:64,0]=q_hi [0:64,1]=q_lo, rows 64:128
              duplicate rows 0:64 (partition-shifted gpsimd copies; hi
              copies ride the ACT engine where it would otherwise idle).
  - scoresT[t,s] via ONE fp8 DoubleRow matmul per 256-col block:
      lhsT tiles = (kt8_chunk, kt8_chunk)  [dim-1 stride-0 broadcast]
      rhs  tiles = ([q_hi;q_hi], [q_lo;q_lo])
      => (k_hi+k_lo)^T (q_hi+q_lo) = full-precision k^T q at 0.5 cycles/col
    (the 64 dead contraction rows of the fp16 version carry the lo residuals,
    and DoubleRow halves the per-column cost: 4x fewer PE cycles vs fp16 pair)
  - exp via ACT PSUM -> SBUF fp16 (scale=1/sqrt(K) folded; softmax
    max-subtraction skipped: logits are O(1) for this problem).
  - AV in fp16: out[s, 0:128] and the softmax denominator in one PSUM
    accumulation: lhsT = expT chunk [t,128s], rhs = [V | ones] [t, 129].
  - evict: out = psum[:, 0:V] * (1/denom) via DVE, DMA to DRAM.
  - schedule (ACT exp is the global bottleneck at ~134us busy; everything
    aims to keep its stream dense): mini-start (project only the leading
    128/256 columns, needing just x-tiles 0-1, so the first exp lands ~8us
    in), then a progressive per-4-tile loop (transpose -> qk1(pair0, g) ->
    lag-fed score chunks of the first two groups), then prefill +
    fine-interleaved v_proj, then a steady loop interleaving one AV
    half-chain between score chunk-groups with pair-1 projection slices
    spread across the first bodies; the two static groups' AV is deferred
    past the congested E-entry. Tail AV runs at quarter grain to pipeline
    behind the last exps.
Host side: shard inputs, run SPMD on 8 cores, reassemble (H,B,S,V).
"""

import sys

sys.path.insert(0, "/opt/trn_rl_repo")

import math
from contextlib import ExitStack

import numpy as np

import concourse.bass as bass
import concourse.mybir as mybir
import concourse.tile as tile
from concourse import bacc
from concourse.masks import make_identity

F32 = mybir.dt.float32
F16 = mybir.dt.float16
F8 = mybir.dt.float8e4
DRMODE = mybir.MatmulPerfMode.DoubleRow
ALU = mybir.AluOpType


def build_attention_nc(S=2048, I=1024, K=64, V=128, HPC=4, reps=1, tune=None):
    """Build the single-core Bass program (SPMD: same program on all cores)."""
    assert S % 512 == 0 and I % 128 == 0 and V == 128 and K == 64
    assert HPC % 2 == 0
    NSG = S // 512  # s groups of 512 queries
    NST = S // 128  # 128-row tiles (both s and t)
    NCI = I // 128  # contraction chunks for projections
    NPAIR = HPC // 2
    SCALE = 1.0 / math.sqrt(K)

    nc = bacc.Bacc("TRN2", target_bir_lowering=False)
    x = nc.dram_tensor("x", [S, I], F16, kind="ExternalInput")
    mq = nc.dram_tensor("mq", [HPC, I, K], F16, kind="ExternalInput")
    mk = nc.dram_tensor("mk", [HPC, I, K], F16, kind="ExternalInput")
    mv = nc.dram_tensor("mv", [HPC, I, V], F16, kind="ExternalInput")
    out = nc.dram_tensor("out", [HPC, S, V], F32, kind="ExternalOutput")

    tune = dict(tune or {})
    with tile.TileContext(nc) as tc:
        for rep in range(reps):
            _emit_rep(nc, tc, rep, x, mq, mk, mv, out,
                      S, I, K, V, HPC, NSG, NST, NCI, NPAIR, SCALE, tune)
    nc.compile()
    return nc


def _emit_rep(nc, tc, rep, x, mq, mk, mv, out,
              S, I, K, V, HPC, NSG, NST, NCI, NPAIR, SCALE, tune):
    T = tune.get
    ECH = T("ech", 2)    # score chunks per psc tile / ACT exp op
    AHEAD = T("ahead", 4)
    TPK = T("tpack", 8)  # transposes packed per psum tile/eviction

    with ExitStack() as persist_ctx:
        persist = persist_ctx.enter_context(
            tc.tile_pool(name=f"persist{rep}", bufs=1)
        )

        # ---------------- persistent SBUF tensors ----------------
        ident32 = persist.tile([128, 128], F32, tag="ident32")
        ident = persist.tile([128, 128], F16, tag="ident")
        xT = persist.tile([128, NCI, S], F16, tag="xT")  # x transposed
        qt8 = [persist.tile([128, 2, S], F8, tag=f"qt{h}", name=f"qt{rep}_{h}")
               for h in range(HPC)]
        kt8 = [persist.tile([128, S], F8, tag=f"kt{h}", name=f"kt{rep}_{h}")
               for h in range(HPC)]
        vsb = persist.tile([128, NST, HPC, V + 1], F16, tag="vsb")
        mqp = [persist.tile([128, NCI, 128], F16, tag=f"mqp{p}", name=f"mqp{rep}_{p}") for p in range(NPAIR)]
        mkp = [persist.tile([128, NCI, 128], F16, tag=f"mkp{p}", name=f"mkp{rep}_{p}") for p in range(NPAIR)]
        mvp = persist.tile([128, NCI, HPC * V], F16, tag="mvp")

        # pool stack order (LIFO closes): pproj -> att(psc, exp0, outp, recp)
        # -> stage(xbig) -> ptr ; ptr and stage close after the progressive
        # start, then expp/pav join att_ctx.
        psum_ctx = ExitStack()
        pproj = psum_ctx.enter_context(
            tc.tile_pool(name=f"pproj{rep}", bufs=T("pproj", 2), space="PSUM")
        )
        att_ctx = ExitStack()
        psc = att_ctx.enter_context(
            tc.tile_pool(name=f"psc{rep}", bufs=T("psc", 1), space="PSUM")
        )
        exp0_pool = att_ctx.enter_context(
            tc.tile_pool(name=f"exp0{rep}", bufs=1)
        )
        outp = att_ctx.enter_context(tc.tile_pool(name=f"outp{rep}", bufs=T("outp", 4)))
        recp = att_ctx.enter_context(tc.tile_pool(name=f"recp{rep}", bufs=T("recp", 4)))
        stage_ctx = ExitStack()
        stage = stage_ctx.enter_context(tc.tile_pool(name=f"stage{rep}", bufs=1))
        xbig = stage.tile([128, NST, I], F16, tag="xbig")
        ptr_ctx = ExitStack()
        ptr = ptr_ctx.enter_context(
            tc.tile_pool(name=f"ptr{rep}", bufs=T("ptr", 1), space="PSUM")
        )
        xr = x.rearrange("(st p) i -> p st i", p=128)

        # ------------- phase 0: DMAs, ordered for earliest first-scores ----
        # x tiles 0:4 -> pair0 weights -> x 4:8 -> pair1 weights -> x 8:12
        # -> V weights -> x 12:16
        def dma_x(lo, hi):
            for u in range(lo, hi):
                nc.sync.dma_start(xbig[:, u : u + 1, :], xr[:, u : u + 1, :])

        def dma_wpair(p):
            for j in range(2):
                h = 2 * p + j
                nc.sync.dma_start(
                    mqp[p][:, :, j * K : (j + 1) * K],
                    mq[h].rearrange("(c i) k -> i c k", i=128),
                )
                nc.sync.dma_start(
                    mkp[p][:, :, j * K : (j + 1) * K],
                    mk[h].rearrange("(c i) k -> i c k", i=128),
                )

        dma_x(0, 4)
        dma_wpair(0)
        dma_x(4, 8)
        dma_wpair(1)
        dma_x(8, 12)
        for h in range(HPC):
            nc.sync.dma_start(
                mvp[:, :, h * V : (h + 1) * V],
                mv[h].rearrange("(c i) v -> i c v", i=128),
            )
        dma_x(12, NST)

        # init ops after DMA issue so they overlap the transfers
        make_identity(nc, ident32)
        nc.vector.tensor_copy(ident[:], ident32[:])
        nc.vector.memset(vsb[:, :, :, V : V + 1], 1.0)

        # ------------- emit helpers -------------
        def emit_tr(st):
            for ci0 in range(0, NCI, TPK):
                pt = ptr.tile([128, TPK, 128], F16, tag="pt", name=f"pt{rep}_{st}_{ci0}")
                for j in range(TPK):
                    ci = ci0 + j
                    nc.tensor.transpose(
                        pt[:, j, :], xbig[:, st, ci * 128 : (ci + 1) * 128], ident[:]
                    )
                nc.vector.tensor_copy(
                    xT[:, ci0 : ci0 + TPK, st * 128 : (st + 1) * 128], pt[:]
                )

        def emit_qk1_mms(p, sg, psq, psk, ci0, ci1):
            for ci in range(ci0, ci1):
                nc.tensor.matmul(
                    psq[:],
                    lhsT=mqp[p][:, ci, :],
                    rhs=xT[:, ci, sg * 512 : (sg + 1) * 512],
                    start=(ci == 0),
                    stop=(ci == NCI - 1),
                )
                nc.tensor.matmul(
                    psk[:],
                    lhsT=mkp[p][:, ci, :],
                    rhs=xT[:, ci, sg * 512 : (sg + 1) * 512],
                    start=(ci == 0),
                    stop=(ci == NCI - 1),
                )

        def emit_qk1_alloc(p, sg):
            psq = pproj.tile([128, 512], F32, tag="pp", name=f"psq{rep}_{p}_{sg}")
            psk = pproj.tile([128, 512], F32, tag="pp", name=f"psk{rep}_{p}_{sg}")
            return psq, psk

        def emit_q_evict(p, sg, psq, c0=0, c1=512):
            blk = slice(sg * 512 + c0, sg * 512 + c1)
            # hi copies on ACT for pair 0 (ACT idles during the progressive
            # start; Copy and Exp share activation table 0 so no reloads).
            # Pair 1 evicts during the ACT-saturated steady phase -> DVE.
            hi_copy = nc.scalar.copy if (p == 0 and T("his_act", 1)) else nc.vector.tensor_copy
            for j in range(2):
                h = 2 * p + j
                rows = slice(j * 64, (j + 1) * 64)
                # q: hi, lo at partitions 0:64; duplicate to 64:128 on gpsimd
                hi_copy(qt8[h][0:64, 0, blk], psq[rows, c0:c1])
                nc.vector.scalar_tensor_tensor(
                    qt8[h][0:64, 1, blk], psq[rows, c0:c1], 1.0,
                    qt8[h][0:64, 0, blk], ALU.mult, ALU.subtract,
                )
                nc.gpsimd.tensor_copy(qt8[h][64:128, :, blk], qt8[h][0:64, :, blk])

        def emit_k_evict(p, sg, psk, c0=0, c1=512):
            blk = slice(sg * 512 + c0, sg * 512 + c1)
            hi_copy = nc.scalar.copy if (p == 0 and T("his_act", 1)) else nc.vector.tensor_copy
            for j in range(2):
                h = 2 * p + j
                rows = slice(j * 64, (j + 1) * 64)
                # k: hi at partitions 0:64, lo at 64:128 (shifted DVE write)
                hi_copy(kt8[h][0:64, blk], psk[rows, c0:c1])
                nc.vector.scalar_tensor_tensor(
                    kt8[h][64:128, blk], psk[rows, c0:c1], 1.0,
                    kt8[h][0:64, blk], ALU.mult, ALU.subtract,
                )

        def emit_qk1_evict(p, sg, psq, psk):
            emit_q_evict(p, sg, psq)
            emit_k_evict(p, sg, psk)

        def emit_proj_chain(pack, psum, sg, c0=0, c1=512):
            for ci in range(NCI):
                nc.tensor.matmul(
                    psum[:, c0:c1],
                    lhsT=pack[:, ci, :],
                    rhs=xT[:, ci, sg * 512 + c0 : sg * 512 + c1],
                    start=(ci == 0),
                    stop=(ci == NCI - 1),
                )

        def emit_qk1(p, sg):
            # one side's chain first, then its eviction overlapping the other
            # chain. k-first unblocks the lag-feed score chunks of earlier
            # s-groups (they need new kt blocks but old qt blocks).
            psq, psk = emit_qk1_alloc(p, sg)
            if T("kfirst", 1):
                emit_proj_chain(mkp[p], psk, sg)
                emit_k_evict(p, sg, psk)
                emit_proj_chain(mqp[p], psq, sg)
                emit_q_evict(p, sg, psq)
            else:
                emit_proj_chain(mqp[p], psq, sg)
                emit_q_evict(p, sg, psq)
                emit_proj_chain(mkp[p], psk, sg)
                emit_k_evict(p, sg, psk)

        def emit_v_proj(tt0, tt1):
            for tt in range(tt0, tt1):
                psv = pproj.tile([128, HPC * V], F32, tag="pp", name=f"psv{rep}_{tt}")
                for ci in range(NCI):
                    nc.tensor.matmul(
                        psv[:],
                        lhsT=xT[:, ci, tt * 128 : (tt + 1) * 128],
                        rhs=mvp[:, ci, :],
                        start=(ci == 0),
                        stop=(ci == NCI - 1),
                    )
                # single copy (on ACT it fills prefill-phase idle; Copy
                # shares the Exp activation table -> no reload)
                vcp = nc.scalar.copy if T("vsb_act", 1) else nc.vector.tensor_copy
                vcp(
                    vsb[:, tt, :, 0:V],
                    psv[:].rearrange("p (h v) -> p h v", h=HPC),
                )

        _tagalt = [0]

        def emit_scores_chunkgrp(h, sg, ex, c0, ne, s0=0, s1=512, tag=None):
            """DR scores for chunks [c0, c0+ne), s-window [s0, s1), + one
            ACT exp over them. psc has two tags (3- and 2-chunk capacity,
            5 PSUM banks total) so most groups run 6 exp ops, not 8."""
            if tag is None:
                tag = "psA" if _tagalt[0] % 2 == 0 else "psB"
                _tagalt[0] += 1
            cap = 2
            assert ne <= cap
            ps = psc.tile([128, cap, 512], F32, tag=tag,
                          name=f"ps{rep}_{h}_{sg}_{c0}_{s0}")
            for cj in range(ne):
                c = c0 + cj
                klhsT = kt8[h][:, c * 128 : (c + 1) * 128]
                klhsT = klhsT.unsqueeze(1).broadcast_to([128, 2, 128])
                for w0 in range(s0, s1, 256):
                    off = sg * 512 + w0
                    nc.tensor.matmul(
                        ps[:, cj, w0 : w0 + 256],
                        lhsT=klhsT,
                        rhs=qt8[h][:, :, off : off + 256],
                        start=True,
                        stop=True,
                        perf_mode=DRMODE,
                    )
            nc.scalar.activation(
                ex[:, c0 : c0 + ne, s0:s1], ps[:, 0:ne, s0:s1],
                mybir.ActivationFunctionType.Exp, scale=SCALE,
            )

        # 6-op full-group plan: (c0, ne, tag); the "last" variant ends in a
        # 1-chunk op so the final AV pieces wait on the smallest possible op
        GPLAN = [(c0, 2, "psA" if (c0 // 2) % 2 == 0 else "psB")
                 for c0 in range(0, NST, 2)]
        GPLAN_LAST = GPLAN

        def emit_scores_exp(h, sg, ex):
            for c0, ne, tag in GPLAN:
                emit_scores_chunkgrp(h, sg, ex, c0, ne, tag=tag)

        def emit_av_half(h, sg, ex, stl, half, po, npieces=2, rot=0):
            # rot staggers the accumulation order (chunk rot..rot+15 mod 16)
            # so in the tail each sub's LAST chunk differs and only one sub
            # trails the final exp op
            soff = stl * 128
            step = NST // npieces
            for i in range(half * step, (half + 1) * step):
                c = (rot + i) % NST
                nc.tensor.matmul(
                    po[:],
                    lhsT=ex[:, c, soff : soff + 128],
                    rhs=vsb[:, c, h, 0 : V + 1],
                    start=(i == 0),
                    stop=(i == NST - 1),
                )
            if half != npieces - 1:
                return
            rec = recp.tile([128, 1], F32, tag="rec", name=f"rec{rep}_{h}_{sg}_{stl}")
            nc.vector.reciprocal(rec[:], po[:, V : V + 1])
            ob = outp.tile([128, V], F32, tag="ob", name=f"ob{rep}_{h}_{sg}_{stl}")
            nc.vector.tensor_scalar_mul(ob[:], po[:, 0:V], rec[:])
            row0 = sg * 512 + stl * 128
            nc.sync.dma_start(out[h, row0 : row0 + 128, :], ob[:])

        def emit_av_sub(h, sg, ex, stl):
            po = pav.tile([128, V + 1], F32, tag="po", name=f"po{rep}_{h}_{sg}_{stl}")
            emit_av_half(h, sg, ex, stl, 0, po)
            emit_av_half(h, sg, ex, stl, 1, po)

        # ------------- schedule -------------
        seq = [(h, sg) for h in range(HPC) for sg in range(NSG)]
        NSTAT = T("nstat", 3)  # static prefill ex tiles usable during B
        ex_tiles = {}
        for j in range(NSTAT):
            exj = exp0_pool.tile([128, NST, 512], F16, tag=f"ex{j}",
                                 name=f"ex{rep}_s{j}")
            ex_tiles[seq[j]] = exj

        # B: progressive start: per 4-st group: transpose, qk1(pair0, g),
        # then lag-feeds: chunks of static group j at lag j+BLAG (consuming
        # the previous iteration's projections avoids waiting on the fresh
        # eviction chain; group 0 starts at lag 0 for the earliest exp).
        BLAG = T("blag", 0)
        done = {}  # group idx -> chunks emitted
        for g in range(NSG):
            if g == 0 and T("ministart", 1):
                # fast first-exp: project only the leading columns (x-tiles
                # 0-1), score chunk 0 over s 0:256, then the block's rest
                ex0 = ex_tiles[seq[0]]
                emit_tr(0)
                emit_tr(1)
                psq, psk = emit_qk1_alloc(0, 0)
                emit_proj_chain(mkp[0], psk, 0, 0, 128)
                emit_k_evict(0, 0, psk, 0, 128)
                emit_proj_chain(mqp[0], psq, 0, 0, 256)
                emit_q_evict(0, 0, psq, 0, 256)
                emit_scores_chunkgrp(0, 0, ex0, 0, 1, 0, 256)
                emit_tr(2)
                emit_tr(3)
                emit_proj_chain(mkp[0], psk, 0, 128, 512)
                emit_k_evict(0, 0, psk, 128, 512)
                emit_proj_chain(mqp[0], psq, 0, 256, 512)
                emit_q_evict(0, 0, psq, 256, 512)
                emit_scores_chunkgrp(0, 0, ex0, 0, 1, 256, 512)
                emit_scores_chunkgrp(0, 0, ex0, 1, 1)
                for c0 in range(2, 4, ECH):
                    emit_scores_chunkgrp(0, 0, ex0, c0, min(ECH, 4 - c0))
                done[0] = 4
                continue
            for st in range(4 * g, 4 * g + 4):
                emit_tr(st)
            emit_qk1(0, g)
            for j in range(NSTAT):
                gl = g - j - (BLAG if (j, g) != (0, 0) else 0)
                gl = min(gl, g)
                if gl < 0 or seq[j][1] > g:
                    continue  # q-block for that group not projected yet
                for c0 in range(done.get(j, 4 * gl), 4 * gl + 4, ECH):
                    emit_scores_chunkgrp(*seq[j], ex_tiles[seq[j]], c0,
                                         min(ECH, 4 * gl + 4 - c0))
                done[j] = 4 * gl + 4

        # C: close transpose PSUM + xbig staging; open steady-state pools
        ptr_ctx.close()
        stage_ctx.close()
        expp = att_ctx.enter_context(
            tc.tile_pool(name=f"expp{rep}", bufs=T("expp", AHEAD))
        )
        pav = att_ctx.enter_context(
            tc.tile_pool(name=f"pav{rep}", bufs=T("pav", 2), space="PSUM")
        )

        # D: finish static groups' tails, prefill groups NSTAT..AHEAD-1, with
        # v_proj tiles interleaved finely between score chunk-groups.
        dscg = []
        for j in range(NSTAT):
            for c0 in range(done.get(j, 0), NST, 2):
                dscg.append((*seq[j], ex_tiles[seq[j]], c0, 2, None))
        for h, sg in [seq[k] for k in range(NSTAT, AHEAD)]:
            nex = expp.tile([128, NST, 512], F16, tag="ex", name=f"ex{rep}_{h}_{sg}")
            ex_tiles[(h, sg)] = nex
            dscg += [(h, sg, nex, c0, ne, tag) for c0, ne, tag in GPLAN]
        QK1D = T("qk1d", 0)  # pair-1 projections emitted in D (rest in E)
        qk1d_at = {(len(dscg) * (i + 1)) // (QK1D + 1): i for i in range(QK1D)}
        vt = 0
        for i, (h, sg, ex, c0, ne, tag) in enumerate(dscg):
            emit_scores_chunkgrp(h, sg, ex, c0, ne, tag=tag)
            if i in qk1d_at:
                emit_qk1(1, qk1d_at[i])
            vt_goal = (i + 1) * NST // len(dscg)
            if vt_goal > vt:
                emit_v_proj(vt, vt_goal)
                vt = vt_goal
        emit_v_proj(vt, NST)

        # E: steady loop, fine round-robin: one AV half-chain (and a slice of
        # the pair-1 projections for k < NSG) between consecutive score
        # chunk-groups so PE never parks a long chain behind a psc-slot wait.
        NDEFER = T("ndefer", 2)  # static groups whose AV defers past E entry
        deferred = []  # (h, sg, ex, stl) subs from static groups
        for k, (h, sg) in enumerate(seq):
            ex = ex_tiles.pop((h, sg))
            nk = seq[k + AHEAD] if k + AHEAD < len(seq) else None
            qpieces = []
            if T("qk1d", 0) <= k < NSG:
                psq, psk = emit_qk1_alloc(1, k)
                step = (NCI + 3) // 4
                qpieces = [("mm", psq, psk, c, min(NCI, c + step))
                           for c in range(0, NCI, step)]
                qpieces.append(("evict", psq, psk, 0, 0))
            if nk is not None:
                nex = expp.tile([128, NST, 512], F16, tag="ex",
                                name=f"ex{rep}_{nk[0]}_{nk[1]}")
                ex_tiles[nk] = nex
            # static groups (never rewritten) can defer their AV past the
            # congested E-entry bodies (which also carry pair-1 projections)
            subs = [(h, sg, ex, stl) for stl in range(4)]
            if k < min(NDEFER, NSTAT):
                deferred += subs
                subs = []
            if k >= NSG and deferred:
                subs.append(deferred.pop(0))
                if k + AHEAD >= len(seq) and deferred:
                    subs.append(deferred.pop(0))
            # last two bodies: quarter-grain AV pieces so the final AV
            # chains pipeline behind the tail of the exp stream
            npieces = 4 if k >= len(seq) - 2 else 2
            halves = [(s, hf) for s in subs for hf in range(npieces)]
            pos = {}
            done_h = 0
            plan = GPLAN_LAST if nk == seq[-1] else GPLAN
            NCG = len(plan) if nk is not None else 6
            for g in range(NCG):
                if nk is not None:
                    c0, ne, tag = plan[g]
                    emit_scores_chunkgrp(nk[0], nk[1], nex, c0, ne, tag=tag)
                goal = (len(halves) * (g + 1) + NCG - 1) // NCG
                while done_h < goal:
                    (hh, ssg, eex, stl), hf = halves[done_h]
                    if hf == 0:
                        pos[(hh, ssg, stl)] = pav.tile(
                            [128, V + 1], F32, tag="po",
                            name=f"po{rep}_{hh}_{ssg}_{stl}")
                    rot = 4 * stl if k == len(seq) - 1 else 0
                    emit_av_half(hh, ssg, eex, stl, hf,
                                 pos[(hh, ssg, stl)], npieces, rot)
                    done_h += 1
                if qpieces:
                    kind, psq, psk, c0, c1 = qpieces.pop(0)
                    if kind == "mm":
                        emit_qk1_mms(1, k, psq, psk, c0, c1)
                    else:
                        emit_qk1_evict(1, k, psq, psk)
        att_ctx.close()
        psum_ctx.close()


_NC_CACHE = {}

DEFAULT_TUNE = {"tpack": 8, "ahead": 5, "expp": 5, "ech": 2, "nstat": 2, "kfirst": 0, "vsb_act": 0, "qk1d": 1, "ministart": 1}


def _install_neff_cache():
    """Persistent on-disk NEFF cache keyed on BIR hash."""
    try:
        import hashlib
        import os
        import shutil

        import concourse.bass_utils as bu
        from concourse import bass2jax

        if getattr(bu.compile_bir_kernel, "_is_cached_wrapper", False):
            return
        orig = bu.compile_bir_kernel
        cache_dir = "/root/neffcache"

        def cached(bir_json, tmpdir, neff_name="file.neff"):
            try:
                h = hashlib.sha256(bir_json).hexdigest()[:24]
                cpath = os.path.join(cache_dir, f"{h}.neff")
                if os.path.exists(cpath):
                    dst = os.path.join(tmpdir, neff_name)
                    shutil.copy(cpath, dst)
                    return dst
                p = orig(bir_json, tmpdir, neff_name)
                os.makedirs(cache_dir, exist_ok=True)
                shutil.copy(p, cpath)
                return p
            except OSError:
                return orig(bir_json, tmpdir, neff_name)

        cached._is_cached_wrapper = True
        bu.compile_bir_kernel = cached
        bass2jax.compile_bir_kernel = cached
    except Exception:
        pass


def _get_nc():
    if "nc" not in _NC_CACHE:
        _NC_CACHE["nc"] = build_attention_nc(tune=DEFAULT_TUNE)
    return _NC_CACHE["nc"]


def run_sharded(x, Mq, Mk, Mv, **spmd_kwargs):
    """Shard inputs over 8 cores, run, reassemble. Returns (out, BassKernelResults)."""
    _install_neff_cache()
    from concourse.bass_utils import run_bass_kernel_spmd

    B, S, I = x.shape
    H = Mq.shape[0]
    V = Mv.shape[-1]
    HPC = H // 2  # 4 heads per core, 2 head groups
    # fp16 host pre-cast (same rounding the device DVE cast applied before)
    x = np.asarray(x, dtype=np.float16)
    Mq = np.asarray(Mq, dtype=np.float16)
    Mk = np.asarray(Mk, dtype=np.float16)
    Mv = np.asarray(Mv, dtype=np.float16)

    in_maps = []
    for c in range(8):
        b, hg = c // 2, c % 2
        hs = slice(hg * HPC, (hg + 1) * HPC)
        in_maps.append(
            {
                "x": np.ascontiguousarray(x[b]),
                "mq": np.ascontiguousarray(Mq[hs, 0]),
                "mk": np.ascontiguousarray(Mk[hs, 0]),
                "mv": np.ascontiguousarray(Mv[hs, 0]),
            }
        )

    nc = _get_nc()
    br = run_bass_kernel_spmd(nc, in_maps, list(range(8)), **spmd_kwargs)

    outf = np.empty((H, B, S, V), dtype=np.float32)
    for c in range(8):
        b, hg = c // 2, c % 2
        outf[hg * HPC : (hg + 1) * HPC, b] = br.results[c]["out"]
    return outf, br


def kernel(x, Mq, Mk, Mv):
    """Full inputs -> full output (H, B, S, V). Shards over 8 NeuronCores."""
    out, _ = run_sharded(x, Mq, Mk, Mv)
    return out


# revision 45
# speedup vs baseline: 1.0257x; 1.0257x over previous
"""Trainium2 Bass kernel for nn_AttentionBlock (multi-head attention block).

Reference computation (fp32):
    q = einsum('bsi,hbik->hbsk', x, Mq)   # Mq: (H,1,I,K) broadcast over b
    k = einsum('bsi,hbik->hbsk', x, Mk)
    v = einsum('bsi,hbiv->hbsv', x, Mv)
    scores  = einsum('hbsk,hbtk->hbst', q, k) / sqrt(K)
    weights = softmax(scores, axis=-1)
    out     = einsum('hbst,hbtv->hbsv', weights, v)   # (H,B,S,V)

Sharding: 8 cores = 4 batches x 2 head-groups (4 heads each). Attention is
independent per (batch, head) so no cross-core communication is needed.

Per-core kernel design (one batch b, 4 heads):
  - x and the weights are pre-cast to fp16 on the host (same rounding the
    device DVE cast would apply), halving input DMA bytes; weights DMA
    directly into their packed SBUF layouts.
  - xT = x.T via PE transposes in fp16  [I on partitions]
  - QT/KT projections with two heads packed per matmul (lhsT = [Mq_h|Mq_h'],
    128 cols) in fp16 -> PSUM fp32.
  - Q/K evicted as fp8e4 hi/lo pairs (hi = fp8(q), lo = fp8(q - hi)):
      kt8[h]: [128, S] fp8, partitions 0:64 = k_hi, 64:128 = k_lo
      qt8[h]: [128, 2, S] fp8, [0:64,0]=q_hi [0:64,1]=q_lo, rows 64:128
              duplicate rows 0:64 (partition-shifted gpsimd copies; hi
              copies ride the ACT engine where it would otherwise idle).
  - scoresT[t,s] via ONE fp8 DoubleRow matmul per 256-col block:
      lhsT tiles = (kt8_chunk, kt8_chunk)  [dim-1 stride-0 broadcast]
      rhs  tiles = ([q_hi;q_hi], [q_lo;q_lo])
      => (k_hi+k_lo)^T (q_hi+q_lo) = full-precision k^T q at 0.5 cycles/col
    (the 64 dead contraction rows of the fp16 version carry the lo residuals,
    and DoubleRow halves the per-column cost: 4x fewer PE cycles vs fp16 pair)
  - exp via ACT PSUM -> SBUF fp16 (scale=1/sqrt(K) folded; softmax
    max-subtraction skipped: logits are O(1) for this problem).
  - AV in fp16: out[s, 0:128] and the softmax denominator in one PSUM
    accumulation: lhsT = expT chunk [t,128s], rhs = [V | ones] [t, 129].
  - evict: out = psum[:, 0:V] * (1/denom) via DVE, DMA to DRAM.
  - schedule (ACT exp is the global bottleneck at ~134us busy; everything
    aims to keep its stream dense): mini-start (project only the leading
    128/256 columns, needing just x-tiles 0-1, so the first exp lands ~8us
    in), then a progressive per-4-tile loop (transpose -> qk1(pair0, g) ->
    lag-fed score chunks of the first two groups), then prefill +
    fine-interleaved v_proj, then a steady loop interleaving one AV
    half-chain between score chunk-groups with pair-1 projection slices
    spread across the first bodies; the two static groups' AV is deferred
    past the congested E-entry. Tail AV runs at quarter grain to pipeline
    behind the last exps.
Host side: shard inputs, run SPMD on 8 cores, reassemble (H,B,S,V).
"""

import sys

sys.path.insert(0, "/opt/trn_rl_repo")

import math
from contextlib import ExitStack

import numpy as np

import concourse.bass as bass
import concourse.mybir as mybir
import concourse.tile as tile
from concourse import bacc
from concourse.masks import make_identity

F32 = mybir.dt.float32
F16 = mybir.dt.float16
F8 = mybir.dt.float8e4
DRMODE = mybir.MatmulPerfMode.DoubleRow
ALU = mybir.AluOpType


def build_attention_nc(S=2048, I=1024, K=64, V=128, HPC=4, reps=1, tune=None):
    """Build the single-core Bass program (SPMD: same program on all cores)."""
    assert S % 512 == 0 and I % 128 == 0 and V == 128 and K == 64
    assert HPC % 2 == 0
    NSG = S // 512  # s groups of 512 queries
    NST = S // 128  # 128-row tiles (both s and t)
    NCI = I // 128  # contraction chunks for projections
    NPAIR = HPC // 2
    SCALE = 1.0 / math.sqrt(K)

    nc = bacc.Bacc("TRN2", target_bir_lowering=False)
    x = nc.dram_tensor("x", [S, I], F16, kind="ExternalInput")
    mq = nc.dram_tensor("mq", [HPC, I, K], F16, kind="ExternalInput")
    mk = nc.dram_tensor("mk", [HPC, I, K], F16, kind="ExternalInput")
    mv = nc.dram_tensor("mv", [HPC, I, V], F16, kind="ExternalInput")
    out = nc.dram_tensor("out", [HPC, S, V], F32, kind="ExternalOutput")

    tune = dict(tune or {})
    with tile.TileContext(nc) as tc:
        for rep in range(reps):
            _emit_rep(nc, tc, rep, x, mq, mk, mv, out,
                      S, I, K, V, HPC, NSG, NST, NCI, NPAIR, SCALE, tune)
    nc.compile()
    return nc


def _emit_rep(nc, tc, rep, x, mq, mk, mv, out,
              S, I, K, V, HPC, NSG, NST, NCI, NPAIR, SCALE, tune):
    T = tune.get
    ECH = T("ech", 2)    # score chunks per psc tile / ACT exp op
    AHEAD = T("ahead", 4)
    TPK = T("tpack", 8)  # transposes packed per psum tile/eviction

    with ExitStack() as persist_ctx:
        persist = persist_ctx.enter_context(
            tc.tile_pool(name=f"persist{rep}", bufs=1)
        )

        # ---------------- persistent SBUF tensors ----------------
        ident32 = persist.tile([128, 128], F32, tag="ident32")
        ident = persist.tile([128, 128], F16, tag="ident")
        xT = persist.tile([128, NCI, S], F16, tag="xT")  # x transposed
        qt8 = [persist.tile([128, 2, S], F8, tag=f"qt{h}", name=f"qt{rep}_{h}")
               for h in range(HPC)]
        kt8 = [persist.tile([128, S], F8, tag=f"kt{h}", name=f"kt{rep}_{h}")
               for h in range(HPC)]
        vsb = persist.tile([128, NST, HPC, V + 1], F16, tag="vsb")
        mqp = [persist.tile([128, NCI, 128], F16, tag=f"mqp{p}", name=f"mqp{rep}_{p}") for p in range(NPAIR)]
        mkp = [persist.tile([128, NCI, 128], F16, tag=f"mkp{p}", name=f"mkp{rep}_{p}") for p in range(NPAIR)]
        mvp = persist.tile([128, NCI, HPC * V], F16, tag="mvp")

        # pool stack order (LIFO closes): pproj -> att(psc, exp0, outp, recp)
        # -> stage(xbig) -> ptr ; ptr and stage close after the progressive
        # start, then expp/pav join att_ctx.
        psum_ctx = ExitStack()
        pproj = psum_ctx.enter_context(
            tc.tile_pool(name=f"pproj{rep}", bufs=T("pproj", 2), space="PSUM")
        )
        att_ctx = ExitStack()
        psc = att_ctx.enter_context(
            tc.tile_pool(name=f"psc{rep}", bufs=T("psc", 1), space="PSUM")
        )
        exp0_pool = att_ctx.enter_context(
            tc.tile_pool(name=f"exp0{rep}", bufs=1)
        )
        outp = att_ctx.enter_context(tc.tile_pool(name=f"outp{rep}", bufs=T("outp", 4)))
        recp = att_ctx.enter_context(tc.tile_pool(name=f"recp{rep}", bufs=T("recp", 4)))
        stage_ctx = ExitStack()
        stage = stage_ctx.enter_context(tc.tile_pool(name=f"stage{rep}", bufs=1))
        xbig = stage.tile([128, NST, I], F16, tag="xbig")
        ptr_ctx = ExitStack()
        ptr = ptr_ctx.enter_context(
            tc.tile_pool(name=f"ptr{rep}", bufs=T("ptr", 1), space="PSUM")
        )
        xr = x.rearrange("(st p) i -> p st i", p=128)

        # ------------- phase 0: DMAs, ordered for earliest first-scores ----
        # x tiles 0:4 -> pair0 weights -> x 4:8 -> pair1 weights -> x 8:12
        # -> V weights -> x 12:16
        def dma_x(lo, hi):
            for u in range(lo, hi):
                nc.sync.dma_start(xbig[:, u : u + 1, :], xr[:, u : u + 1, :])

        def dma_wpair(p):
            for j in range(2):
                h = 2 * p + j
                nc.sync.dma_start(
                    mqp[p][:, :, j * K : (j + 1) * K],
                    mq[h].rearrange("(c i) k -> i c k", i=128),
                )
                nc.sync.dma_start(
                    mkp[p][:, :, j * K : (j + 1) * K],
                    mk[h].rearrange("(c i) k -> i c k", i=128),
                )

        dma_x(0, 4)
        dma_wpair(0)
        dma_x(4, 8)
        dma_wpair(1)
        dma_x(8, 12)
        for h in range(HPC):
            nc.sync.dma_start(
                mvp[:, :, h * V : (h + 1) * V],
                mv[h].rearrange("(c i) v -> i c v", i=128),
            )
        dma_x(12, NST)

        # init ops after DMA issue so they overlap the transfers
        make_identity(nc, ident32)
        nc.vector.tensor_copy(ident[:], ident32[:])
        nc.vector.memset(vsb[:, :, :, V : V + 1], 1.0)

        # ------------- emit helpers -------------
        def emit_tr(st):
            for ci0 in range(0, NCI, TPK):
                pt = ptr.tile([128, TPK, 128], F16, tag="pt", name=f"pt{rep}_{st}_{ci0}")
                for j in range(TPK):
                    ci = ci0 + j
                    nc.tensor.transpose(
                        pt[:, j, :], xbig[:, st, ci * 128 : (ci + 1) * 128], ident[:]
                    )
                nc.vector.tensor_copy(
                    xT[:, ci0 : ci0 + TPK, st * 128 : (st + 1) * 128], pt[:]
                )

        def emit_qk1_mms(p, sg, psq, psk, ci0, ci1):
            for ci in range(ci0, ci1):
                nc.tensor.matmul(
                    psq[:],
                    lhsT=mqp[p][:, ci, :],
                    rhs=xT[:, ci, sg * 512 : (sg + 1) * 512],
                    start=(ci == 0),
                    stop=(ci == NCI - 1),
                )
                nc.tensor.matmul(
                    psk[:],
                    lhsT=mkp[p][:, ci, :],
                    rhs=xT[:, ci, sg * 512 : (sg + 1) * 512],
                    start=(ci == 0),
                    stop=(ci == NCI - 1),
                )

        def emit_qk1_alloc(p, sg):
            psq = pproj.tile([128, 512], F32, tag="pp", name=f"psq{rep}_{p}_{sg}")
            psk = pproj.tile([128, 512], F32, tag="pp", name=f"psk{rep}_{p}_{sg}")
            return psq, psk

        def emit_q_evict(p, sg, psq, c0=0, c1=512):
            blk = slice(sg * 512 + c0, sg * 512 + c1)
            # hi copies on ACT for pair 0 (ACT idles during the progressive
            # start; Copy and Exp share activation table 0 so no reloads).
            # Pair 1 evicts during the ACT-saturated steady phase -> DVE.
            hi_copy = nc.scalar.copy if (p == 0 and T("his_act", 1)) else nc.vector.tensor_copy
            for j in range(2):
                h = 2 * p + j
                rows = slice(j * 64, (j + 1) * 64)
                # q: hi, lo at partitions 0:64; duplicate to 64:128 on gpsimd
                hi_copy(qt8[h][0:64, 0, blk], psq[rows, c0:c1])
                nc.vector.scalar_tensor_tensor(
                    qt8[h][0:64, 1, blk], psq[rows, c0:c1], 1.0,
                    qt8[h][0:64, 0, blk], ALU.mult, ALU.subtract,
                )
                if T("dup_dve", 0):
                    # same-engine dup: no cross-engine semaphore hop in the
                    # eviction -> scores dependency chain
                    nc.vector.tensor_copy(qt8[h][64:128, :, blk],
                                          qt8[h][0:64, :, blk])
                else:
                    nc.gpsimd.tensor_copy(qt8[h][64:128, :, blk],
                                          qt8[h][0:64, :, blk])

        def emit_k_evict(p, sg, psk, c0=0, c1=512):
            blk = slice(sg * 512 + c0, sg * 512 + c1)
            hi_copy = (nc.scalar.copy if (p == 0 and T("his_act", 1)
                       and not T("khis_dve", 0)) else nc.vector.tensor_copy)
            for j in range(2):
                h = 2 * p + j
                rows = slice(j * 64, (j + 1) * 64)
                # k: hi at partitions 0:64, lo at 64:128 (shifted DVE write)
                hi_copy(kt8[h][0:64, blk], psk[rows, c0:c1])
                nc.vector.scalar_tensor_tensor(
                    kt8[h][64:128, blk], psk[rows, c0:c1], 1.0,
                    kt8[h][0:64, blk], ALU.mult, ALU.subtract,
                )

        def emit_qk1_evict(p, sg, psq, psk):
            emit_q_evict(p, sg, psq)
            emit_k_evict(p, sg, psk)

        def emit_proj_chain(pack, psum, sg, c0=0, c1=512):
            for ci in range(NCI):
                nc.tensor.matmul(
                    psum[:, c0:c1],
                    lhsT=pack[:, ci, :],
                    rhs=xT[:, ci, sg * 512 + c0 : sg * 512 + c1],
                    start=(ci == 0),
                    stop=(ci == NCI - 1),
                )

        def emit_qk1(p, sg):
            # one side's chain first, then its eviction overlapping the other
            # chain. k-first unblocks the lag-feed score chunks of earlier
            # s-groups (they need new kt blocks but old qt blocks).
            psq, psk = emit_qk1_alloc(p, sg)
            if T("kfirst", 1):
                emit_proj_chain(mkp[p], psk, sg)
                emit_k_evict(p, sg, psk)
                emit_proj_chain(mqp[p], psq, sg)
                emit_q_evict(p, sg, psq)
            else:
                emit_proj_chain(mqp[p], psq, sg)
                emit_q_evict(p, sg, psq)
                emit_proj_chain(mkp[p], psk, sg)
                emit_k_evict(p, sg, psk)

        def emit_v_proj(tt0, tt1):
            for tt in range(tt0, tt1):
                psv = pproj.tile([128, HPC * V], F32, tag="pp", name=f"psv{rep}_{tt}")
                for ci in range(NCI):
                    nc.tensor.matmul(
                        psv[:],
                        lhsT=xT[:, ci, tt * 128 : (tt + 1) * 128],
                        rhs=mvp[:, ci, :],
                        start=(ci == 0),
                        stop=(ci == NCI - 1),
                    )
                # single copy (on ACT it fills prefill-phase idle; Copy
                # shares the Exp activation table -> no reload)
                vcp = nc.scalar.copy if T("vsb_act", 1) else nc.vector.tensor_copy
                vcp(
                    vsb[:, tt, :, 0:V],
                    psv[:].rearrange("p (h v) -> p h v", h=HPC),
                )

        _tagalt = [0]

        def emit_scores_chunkgrp(h, sg, ex, c0, ne, s0=0, s1=512, tag=None):
            """DR scores for chunks [c0, c0+ne), s-window [s0, s1), + one
            ACT exp over them. psc has two tags (3- and 2-chunk capacity,
            5 PSUM banks total) so most groups run 6 exp ops, not 8."""
            if tag is None:
                tag = "psA" if _tagalt[0] % 2 == 0 else "psB"
                _tagalt[0] += 1
            cap = 2
            assert ne <= cap
            ps = psc.tile([128, cap, 512], F32, tag=tag,
                          name=f"ps{rep}_{h}_{sg}_{c0}_{s0}")
            for cj in range(ne):
                c = c0 + cj
                klhsT = kt8[h][:, c * 128 : (c + 1) * 128]
                klhsT = klhsT.unsqueeze(1).broadcast_to([128, 2, 128])
                for w0 in range(s0, s1, 256):
                    off = sg * 512 + w0
                    nc.tensor.matmul(
                        ps[:, cj, w0 : w0 + 256],
                        lhsT=klhsT,
                        rhs=qt8[h][:, :, off : off + 256],
                        start=True,
                        stop=True,
                        perf_mode=DRMODE,
                    )
            nc.scalar.activation(
                ex[:, c0 : c0 + ne, s0:s1], ps[:, 0:ne, s0:s1],
                mybir.ActivationFunctionType.Exp, scale=SCALE,
            )

        # 6-op full-group plan: (c0, ne, tag); the "last" variant ends in a
        # 1-chunk op so the final AV pieces wait on the smallest possible op
        GPLAN = [(c0, 2, "psA" if (c0 // 2) % 2 == 0 else "psB")
                 for c0 in range(0, NST, 2)]
        GPLAN_LAST = GPLAN

        def emit_scores_exp(h, sg, ex):
            for c0, ne, tag in GPLAN:
                emit_scores_chunkgrp(h, sg, ex, c0, ne, tag=tag)

        def emit_av_half(h, sg, ex, stl, half, po, npieces=2, rot=0):
            # rot staggers the accumulation order (chunk rot..rot+15 mod 16)
            # so in the tail each sub's LAST chunk differs and only one sub
            # trails the final exp op
            soff = stl * 128
            step = NST // npieces
            for i in range(half * step, (half + 1) * step):
                c = (rot + i) % NST
                nc.tensor.matmul(
                    po[:],
                    lhsT=ex[:, c, soff : soff + 128],
                    rhs=vsb[:, c, h, 0 : V + 1],
                    start=(i == 0),
                    stop=(i == NST - 1),
                )
            if half != npieces - 1:
                return
            rec = recp.tile([128, 1], F32, tag="rec", name=f"rec{rep}_{h}_{sg}_{stl}")
            nc.vector.reciprocal(rec[:], po[:, V : V + 1])
            ob = outp.tile([128, V], F32, tag="ob", name=f"ob{rep}_{h}_{sg}_{stl}")
            nc.vector.tensor_scalar_mul(ob[:], po[:, 0:V], rec[:])
            row0 = sg * 512 + stl * 128
            nc.sync.dma_start(out[h, row0 : row0 + 128, :], ob[:])

        def emit_av_sub(h, sg, ex, stl):
            po = pav.tile([128, V + 1], F32, tag="po", name=f"po{rep}_{h}_{sg}_{stl}")
            emit_av_half(h, sg, ex, stl, 0, po)
            emit_av_half(h, sg, ex, stl, 1, po)

        # ------------- schedule -------------
        seq = [(h, sg) for h in range(HPC) for sg in range(NSG)]
        NSTAT = T("nstat", 3)  # static prefill ex tiles usable during B
        ex_tiles = {}
        for j in range(NSTAT):
            exj = exp0_pool.tile([128, NST, 512], F16, tag=f"ex{j}",
                                 name=f"ex{rep}_s{j}")
            ex_tiles[seq[j]] = exj

        # B: progressive start: per 4-st group: transpose, qk1(pair0, g),
        # then lag-feeds: chunks of static group j at lag j+BLAG (consuming
        # the previous iteration's projections avoids waiting on the fresh
        # eviction chain; group 0 starts at lag 0 for the earliest exp).
        BLAG = T("blag", 0)
        done = {}  # group idx -> chunks emitted
        for g in range(NSG):
            if g == 0 and T("ministart", 1):
                # fast first-exp: project only the leading columns (x-tiles
                # 0-1), score chunk 0 over s 0:256, then the block's rest
                ex0 = ex_tiles[seq[0]]
                emit_tr(0)
                emit_tr(1)
                psq, psk = emit_qk1_alloc(0, 0)
                emit_proj_chain(mkp[0], psk, 0, 0, 128)
                emit_k_evict(0, 0, psk, 0, 128)
                emit_proj_chain(mqp[0], psq, 0, 0, 256)
                emit_q_evict(0, 0, psq, 0, 256)
                emit_scores_chunkgrp(0, 0, ex0, 0, 1, 0, 256)
                emit_tr(2)
                emit_tr(3)
                emit_proj_chain(mkp[0], psk, 0, 128, 512)
                emit_k_evict(0, 0, psk, 128, 512)
                emit_proj_chain(mqp[0], psq, 0, 256, 512)
                emit_q_evict(0, 0, psq, 256, 512)
                emit_scores_chunkgrp(0, 0, ex0, 0, 1, 256, 512)
                emit_scores_chunkgrp(0, 0, ex0, 1, 1)
                for c0 in range(2, 4, ECH):
                    emit_scores_chunkgrp(0, 0, ex0, c0, min(ECH, 4 - c0))
                done[0] = 4
                continue
            for st in range(4 * g, 4 * g + 4):
                emit_tr(st)
            emit_qk1(0, g)
            for j in range(NSTAT):
                gl = g - j - (BLAG if (j, g) != (0, 0) else 0)
                gl = min(gl, g)
                if gl < 0 or seq[j][1] > g:
                    continue  # q-block for that group not projected yet
                for c0 in range(done.get(j, 4 * gl), 4 * gl + 4, ECH):
                    emit_scores_chunkgrp(*seq[j], ex_tiles[seq[j]], c0,
                                         min(ECH, 4 * gl + 4 - c0))
                done[j] = 4 * gl + 4

        # C: close transpose PSUM + xbig staging; open steady-state pools
        ptr_ctx.close()
        stage_ctx.close()
        expp = att_ctx.enter_context(
            tc.tile_pool(name=f"expp{rep}", bufs=T("expp", AHEAD))
        )
        pav = att_ctx.enter_context(
            tc.tile_pool(name=f"pav{rep}", bufs=T("pav", 2), space="PSUM")
        )

        # D: finish static groups' tails, prefill groups NSTAT..AHEAD-1, with
        # v_proj tiles interleaved finely between score chunk-groups.
        dscg = []
        for j in range(NSTAT):
            for c0 in range(done.get(j, 0), NST, 2):
                dscg.append((*seq[j], ex_tiles[seq[j]], c0, 2, None))
        for h, sg in [seq[k] for k in range(NSTAT, AHEAD)]:
            nex = expp.tile([128, NST, 512], F16, tag="ex", name=f"ex{rep}_{h}_{sg}")
            ex_tiles[(h, sg)] = nex
            dscg += [(h, sg, nex, c0, ne, tag) for c0, ne, tag in GPLAN]
        QK1D = T("qk1d", 0)  # pair-1 projections emitted in D (rest in E)
        qk1d_at = {(len(dscg) * (i + 1)) // (QK1D + 1): i for i in range(QK1D)}
        vt = 0
        for i, (h, sg, ex, c0, ne, tag) in enumerate(dscg):
            emit_scores_chunkgrp(h, sg, ex, c0, ne, tag=tag)
            if i in qk1d_at:
                emit_qk1(1, qk1d_at[i])
            vt_goal = (i + 1) * NST // len(dscg)
            if vt_goal > vt:
                emit_v_proj(vt, vt_goal)
                vt = vt_goal
        emit_v_proj(vt, NST)

        # E: steady loop, fine round-robin: one AV half-chain (and a slice of
        # the pair-1 projections for k < NSG) between consecutive score
        # chunk-groups so PE never parks a long chain behind a psc-slot wait.
        NDEFER = T("ndefer", 2)  # static groups whose AV defers past E entry
        deferred = []  # (h, sg, ex, stl) subs from static groups
        for k, (h, sg) in enumerate(seq):
            ex = ex_tiles.pop((h, sg))
            nk = seq[k + AHEAD] if k + AHEAD < len(seq) else None
            qpieces = []
            if T("qk1d", 0) <= k < NSG:
                psq, psk = emit_qk1_alloc(1, k)
                step = (NCI + 3) // 4
                qpieces = [("mm", psq, psk, c, min(NCI, c + step))
                           for c in range(0, NCI, step)]
                qpieces.append(("evict", psq, psk, 0, 0))
            if nk is not None:
                nex = expp.tile([128, NST, 512], F16, tag="ex",
                                name=f"ex{rep}_{nk[0]}_{nk[1]}")
                ex_tiles[nk] = nex
            # static groups (never rewritten) can defer their AV past the
            # congested E-entry bodies (which also carry pair-1 projections)
            subs = [(h, sg, ex, stl) for stl in range(4)]
            if k < min(NDEFER, NSTAT):
                deferred += subs
                subs = []
            if k >= NSG and deferred:
                subs.append(deferred.pop(0))
                if k + AHEAD >= len(seq) and deferred:
                    subs.append(deferred.pop(0))
            # last two bodies: quarter-grain AV pieces so the final AV
            # chains pipeline behind the tail of the exp stream
            npieces = 4 if k >= len(seq) - 2 else 2
            halves = [(s, hf) for s in subs for hf in range(npieces)]
            pos = {}
            done_h = 0
            plan = GPLAN_LAST if nk == seq[-1] else GPLAN
            NCG = len(plan) if nk is not None else 6
            for g in range(NCG):
                if nk is not None:
                    c0, ne, tag = plan[g]
                    emit_scores_chunkgrp(nk[0], nk[1], nex, c0, ne, tag=tag)
                goal = (len(halves) * (g + 1) + NCG - 1) // NCG
                while done_h < goal:
                    (hh, ssg, eex, stl), hf = halves[done_h]
                    if hf == 0:
                        pos[(hh, ssg, stl)] = pav.tile(
                            [128, V + 1], F32, tag="po",
                            name=f"po{rep}_{hh}_{ssg}_{stl}")
                    rot = 4 * stl if k == len(seq) - 1 else 0
                    emit_av_half(hh, ssg, eex, stl, hf,
                                 pos[(hh, ssg, stl)], npieces, rot)
                    done_h += 1
                if qpieces:
                    kind, psq, psk, c0, c1 = qpieces.pop(0)
                    if kind == "mm":
                        emit_qk1_mms(1, k, psq, psk, c0, c1)
                    else:
                        emit_qk1_evict(1, k, psq, psk)
        att_ctx.close()
        psum_ctx.close()


_NC_CACHE = {}

DEFAULT_TUNE = {"tpack": 8, "ahead": 5, "expp": 5, "ech": 2, "nstat": 2, "kfirst": 0, "vsb_act": 0, "qk1d": 1, "ministart": 1, "ptr": 2}


def _install_neff_cache():
    """Persistent on-disk NEFF cache keyed on BIR hash."""
    try:
        import hashlib
        import os
        import shutil

        import concourse.bass_utils as bu
        from concourse import bass2jax

        if getattr(bu.compile_bir_kernel, "_is_cached_wrapper", False):
            return
        orig = bu.compile_bir_kernel
        cache_dir = "/root/neffcache"

        def cached(bir_json, tmpdir, neff_name="file.neff"):
            try:
                h = hashlib.sha256(bir_json).hexdigest()[:24]
                cpath = os.path.join(cache_dir, f"{h}.neff")
                if os.path.exists(cpath):
                    dst = os.path.join(tmpdir, neff_name)
                    shutil.copy(cpath, dst)
                    return dst
                p = orig(bir_json, tmpdir, neff_name)
                os.makedirs(cache_dir, exist_ok=True)
                shutil.copy(p, cpath)
                return p
            except OSError:
                return orig(bir_json, tmpdir, neff_name)

        cached._is_cached_wrapper = True
        bu.compile_bir_kernel = cached
        bass2jax.compile_bir_kernel = cached
    except Exception:
        pass


def _get_nc():
    if "nc" not in _NC_CACHE:
        _NC_CACHE["nc"] = build_attention_nc(tune=DEFAULT_TUNE)
    return _NC_CACHE["nc"]


def run_sharded(x, Mq, Mk, Mv, **spmd_kwargs):
    """Shard inputs over 8 cores, run, reassemble. Returns (out, BassKernelResults)."""
    _install_neff_cache()
    from concourse.bass_utils import run_bass_kernel_spmd

    B, S, I = x.shape
    H = Mq.shape[0]
    V = Mv.shape[-1]
    HPC = H // 2  # 4 heads per core, 2 head groups
    # fp16 host pre-cast (same rounding the device DVE cast applied before)
    x = np.asarray(x, dtype=np.float16)
    Mq = np.asarray(Mq, dtype=np.float16)
    Mk = np.asarray(Mk, dtype=np.float16)
    Mv = np.asarray(Mv, dtype=np.float16)

    in_maps = []
    for c in range(8):
        b, hg = c // 2, c % 2
        hs = slice(hg * HPC, (hg + 1) * HPC)
        in_maps.append(
            {
                "x": np.ascontiguousarray(x[b]),
                "mq": np.ascontiguousarray(Mq[hs, 0]),
                "mk": np.ascontiguousarray(Mk[hs, 0]),
                "mv": np.ascontiguousarray(Mv[hs, 0]),
            }
        )

    nc = _get_nc()
    br = run_bass_kernel_spmd(nc, in_maps, list(range(8)), **spmd_kwargs)

    outf = np.empty((H, B, S, V), dtype=np.float32)
    for c in range(8):
        b, hg = c // 2, c % 2
        outf[hg * HPC : (hg + 1) * HPC, b] = br.results[c]["out"]
    return outf, br


def kernel(x, Mq, Mk, Mv):
    """Full inputs -> full output (H, B, S, V). Shards over 8 NeuronCores."""
    out, _ = run_sharded(x, Mq, Mk, Mv)
    return out
